# revision 1
# baseline (speedup 1.0000x reference)
"""BiDirectionalSpatialMamba Trainium2 kernel.

Sharding: data-parallel over B*T=128 sequences -> 16 per core x 8 cores.
Per core, one SPMD bass program with three phases:
  P1: x -> cast/PE-transpose -> x^T -> lin -> gelu -> gi = g@W_ih + bias
      (both branches), gi spilled to DRAM bf16 in block-major layout.
  P2: sequential GRU scan over the 1024 spatial positions, fwd and bwd
      branches interleaved as two independent dependency chains.
      Transposed layout: hidden dim on partitions, 16 seqs on free dim.
  P3: proj([f;b]) + bias, PE transpose back to token-major, +x residual,
      LayerNorm, contiguous DMA out.

All matmul weights bf16 (fp32 weight loads are ~20x slower on the PE);
fp32 accumulation in PSUM; hidden state bf16.
Block-major DRAM layouts keep every DMA contiguous with <=3 AP dims
(this toolchain silently corrupts composite-partition DMA rearranges).
"""
import os
import sys

_HERE = os.path.dirname(os.path.abspath(__file__))
sys.path.insert(0, "/opt/trn_rl_repo")
if _HERE not in sys.path:
    sys.path.insert(0, _HERE)

import numpy as np
import ml_dtypes

import concourse.bass as bass
import concourse.mybir as mybir
import concourse.tile as tile
from concourse.masks import make_identity

# ---------------------------------------------------------------- inlined deps
# (kernel.py must be self-contained: tile-drain workaround, multi-wait
# splitter, and the PJRT SPMD runner are inlined here.)

from concourse.tile_sem_assignment import VectorClock, N_PROCS
from concourse.tile import ScopedClock


def _drain_and_barrier_chunked(self, tick_clock, wait_clock):
    nc = self.nc
    g = tick_clock.global_clock
    vals = [g[p] for p in range(N_PROCS)]
    nz = [p for p in range(N_PROCS) if vals[p] > 0]
    for p in nz:
        cv = [vals[q] if q == p else 0 for q in range(N_PROCS)]
        nop = nc.sync.nop()
        wait_clock.add_sem_waits(nop.ins, ScopedClock({None: VectorClock(cv)}))
    nc.sync.drain()
    nc.all_engine_barrier()
    assert self.sems is not None
    popped = nc._tile_sem_poison_stack.pop()
    assert popped is self._sem_poison
    nc.clear_and_free_semaphores(list(self.sems.allocated().values()))
    nc.all_engine_barrier()


tile.TileContext._drain_and_barrier = _drain_and_barrier_chunked

_SPLIT_SEQ = [0]


def split_multi_waits(nc):
    """This walrus build allows at most ONE sync-wait command per
    instruction; move extra waits onto single-wait NoOps inserted before
    the overloaded instruction (same engine, same bb)."""
    n_split = 0
    for fn in nc.m.functions:
        for bb in fn.blocks:
            insts = list(bb.instructions)
            out = []
            changed = False
            for inst in insts:
                si = inst.sync_info
                if si is not None and si.on_wait and len(si.on_wait) > 1:
                    waits = list(si.on_wait)
                    for w in waits[:-1]:
                        _SPLIT_SEQ[0] += 1
                        nop = mybir.InstNoOp(
                            name=f"zzsplitw-{_SPLIT_SEQ[0]}",
                            engine=inst.engine,
                            sync_info=mybir.SyncInfo(on_wait=[w], on_update=[]),
                        )
                        nc.inst_map[nop.name] = nop
                        out.append(nop)
                        n_split += 1
                    inst.sync_info = mybir.SyncInfo(
                        on_wait=[waits[-1]], on_update=list(si.on_update))
                    changed = True
                out.append(inst)
            if changed:
                bb.instructions = out
    return n_split


class SpmdRunner:
    """Compile once via bass2jax custom-call; run on the 8 cores through
    jax shard_map (axon PJRT)."""

    def __init__(self, nc, n_cores=8):
        import jax
        from jax.sharding import Mesh, PartitionSpec
        from jax.experimental.shard_map import shard_map
        from concourse import bass2jax
        from concourse.bass2jax import _bass_exec_p, install_neuronx_cc_hook
        self.jax = jax
        self.Mesh, self.PartitionSpec = Mesh, PartitionSpec
        install_neuronx_cc_hook()
        self.nc = nc
        self.n_cores = n_cores
        partition_name = (
            nc.partition_id_tensor.name if nc.partition_id_tensor else None)
        in_names, out_names, out_avals, zero_outs = [], [], [], []
        for alloc in nc.m.functions[0].allocations:
            if not isinstance(alloc, mybir.MemoryLocationSet):
                continue
            name = alloc.memorylocations[0].name
            if alloc.kind == "ExternalInput":
                if name != partition_name:
                    in_names.append(name)
            elif alloc.kind == "ExternalOutput":
                out_names.append(name)
                shape = tuple(alloc.tensor_shape)
                dtype = mybir.dt.np(alloc.dtype)
                out_avals.append(jax.core.ShapedArray(shape, dtype))
                zero_outs.append(np.zeros(shape, dtype))
        self.in_names_params = list(in_names)
        n_params = len(in_names)
        n_outs = len(out_avals)
        in_names = in_names + out_names
        if partition_name is not None:
            in_names.append(partition_name)
        self.out_names = out_names
        self.out_avals = out_avals
        self.zero_outs = zero_outs
        donate = tuple(range(n_params, n_params + n_outs))

        def _body(*args):
            operands = list(args)
            if partition_name is not None:
                operands.append(bass2jax.partition_id_tensor())
            outs = _bass_exec_p.bind(
                *operands,
                out_avals=tuple(out_avals),
                in_names=tuple(in_names),
                out_names=tuple(out_names),
                lowering_input_output_aliases=(),
                sim_require_finite=True,
                sim_require_nnan=True,
                nc=nc,
            )
            return tuple(outs)

        devices = jax.devices()[:n_cores]
        mesh = Mesh(np.asarray(devices), ("core",))
        in_specs = (PartitionSpec("core"),) * (n_params + n_outs)
        out_specs = (PartitionSpec("core"),) * len(out_names)
        self.sharded = jax.jit(
            shard_map(_body, mesh=mesh, in_specs=in_specs,
                      out_specs=out_specs, check_rep=False),
            donate_argnums=donate,
            keep_unused=True,
        )

    def _concat_inputs(self, in_maps):
        per_core = [[np.asarray(m[name]) for name in self.in_names_params]
                    for m in in_maps]
        return [np.concatenate([per_core[c][i] for c in range(self.n_cores)],
                               axis=0)
                for i in range(len(self.in_names_params))]

    def _zeros(self):
        return [np.zeros((self.n_cores * z.shape[0], *z.shape[1:]), z.dtype)
                for z in self.zero_outs]

    def run(self, in_maps):
        jax = self.jax
        concat_in = self._concat_inputs(in_maps)
        out_arrs = self.sharded(*concat_in, *self._zeros())
        jax.block_until_ready(out_arrs)
        return [
            {name: np.asarray(out_arrs[i]).reshape(
                self.n_cores, *self.out_avals[i].shape)[c]
             for i, name in enumerate(self.out_names)}
            for c in range(self.n_cores)
        ]

    def time_reps(self, in_maps, reps=8):
        import time as _time
        jax = self.jax
        concat_in = self._concat_inputs(in_maps)
        out = self.sharded(*concat_in, *self._zeros())
        jax.block_until_ready(out)
        times = []
        for _ in range(reps):
            z = self._zeros()
            t0 = _time.perf_counter()
            out = self.sharded(*concat_in, *z)
            jax.block_until_ready(out)
            times.append(_time.perf_counter() - t0)
        return times

    def time_reps_device(self, in_maps, reps=10):
        import time as _time
        jax = self.jax
        from jax.sharding import NamedSharding
        mesh = self.Mesh(np.asarray(jax.devices()[:self.n_cores]), ("core",))
        sh = NamedSharding(mesh, self.PartitionSpec("core"))
        concat_in = [jax.device_put(a, sh) for a in self._concat_inputs(in_maps)]
        outs = self.sharded(*concat_in,
                            *[jax.device_put(z, sh) for z in self._zeros()])
        jax.block_until_ready(outs)
        times = []
        for _ in range(reps):
            t0 = _time.perf_counter()
            outs = self.sharded(*concat_in, *outs)
            jax.block_until_ready(outs)
            times.append(_time.perf_counter() - t0)
        return times

BF16 = mybir.dt.bfloat16
F32 = mybir.dt.float32
AF = mybir.ActivationFunctionType
HID = 256
DIN = 512
G3 = 768
P = 1024
NSEQ = 16          # sequences per core
NC = 8             # cores
BLK = 128          # positions per block (P1 block == scan block)
NBLK = P // BLK    # 8
LN_EPS = 1e-5

_runner_cache = {}


def build_nc():
    nc = bass.Bass(trn_type="TRN2", target_bir_lowering=False, debug=False)

    x_d = nc.dram_tensor("x", [NSEQ, P, HID], F32, kind="ExternalInput")
    linW_d = nc.dram_tensor("linW", [128, 2, 2, DIN], BF16, kind="ExternalInput")
    Wih_d = nc.dram_tensor("Wih", [128, 2, 4, G3], BF16, kind="ExternalInput")
    Whh_d = nc.dram_tensor("Whh", [128, 2, 2, G3], BF16, kind="ExternalInput")
    proj_d = nc.dram_tensor("proj", [128, 4, HID], BF16, kind="ExternalInput")
    gib_d = nc.dram_tensor("gib", [128, 12], F32, kind="ExternalInput")
    linb_d = nc.dram_tensor("linb", [128, 8], F32, kind="ExternalInput")
    projb_d = nc.dram_tensor("projb", [128, 2], F32, kind="ExternalInput")
    lng_d = nc.dram_tensor("lng", [128, HID], F32, kind="ExternalInput")
    lnb_d = nc.dram_tensor("lnb", [128, HID], F32, kind="ExternalInput")
    out_d = nc.dram_tensor("out", [NSEQ, P, HID], F32, kind="ExternalOutput")

    dbg = os.environ.get("KDBG") == "1"
    kind = {"kind": "ExternalOutput"} if dbg else {}
    # block-major intermediates: [q, blk, ch, s, p_local]
    gif_d = nc.dram_tensor("gif", [128, NBLK, 6, NSEQ, BLK], BF16, **kind)
    gibk_d = nc.dram_tensor("gibk", [128, NBLK, 6, NSEQ, BLK], BF16, **kind)
    fT_d = nc.dram_tensor("fT", [128, NBLK, 2, NSEQ, BLK], BF16, **kind)
    bT_d = nc.dram_tensor("bT", [128, NBLK, 2, NSEQ, BLK], BF16, **kind)

    with tile.TileContext(nc) as tc:
        with tc.tile_pool(name="const", bufs=1) as cpool:
            linW = cpool.tile([128, 2, 2, DIN], BF16)
            nc.sync.dma_start(out=linW[:, :, :, :], in_=linW_d[:, :, :, :])
            Wih = cpool.tile([128, 2, 4, G3], BF16)
            nc.sync.dma_start(out=Wih[:, :, :, :], in_=Wih_d[:, :, :, :])
            Whh = cpool.tile([128, 2, 2, G3], BF16)
            nc.sync.dma_start(out=Whh[:, :, :, :], in_=Whh_d[:, :, :, :])
            proj = cpool.tile([128, 4, HID], BF16)
            nc.sync.dma_start(out=proj[:, :, :], in_=proj_d[:, :, :])
            gib = cpool.tile([128, 12], F32)
            nc.sync.dma_start(out=gib[:, :], in_=gib_d[:, :])
            linb = cpool.tile([128, 8], F32)
            nc.sync.dma_start(out=linb[:, :], in_=linb_d[:, :])
            projb = cpool.tile([128, 2], F32)
            nc.sync.dma_start(out=projb[:, :], in_=projb_d[:, :])
            lng = cpool.tile([128, HID], F32)
            nc.sync.dma_start(out=lng[:, :], in_=lng_d[:, :])
            lnb = cpool.tile([128, HID], F32)
            nc.sync.dma_start(out=lnb[:, :], in_=lnb_d[:, :])
            ident_b = cpool.tile([128, 128], BF16)
            make_identity(nc, ident_b[:, :])
            ident_f = cpool.tile([128, 128], F32)
            make_identity(nc, ident_f[:, :])
            hz = cpool.tile([128, 2, NSEQ], BF16)
            nc.gpsimd.memset(hz[:, :, :], 0.0)

            # ---------------- P1: x^T, lin, gelu, gi ----------------
            with (
                tc.tile_pool(name="p1xs", bufs=3) as xsp,
                tc.tile_pool(name="p1xsb", bufs=3) as xsbp,
                tc.tile_pool(name="p1xT", bufs=2) as xTp,
                tc.tile_pool(name="p1gT", bufs=2) as gTp,
                tc.tile_pool(name="p1gi", bufs=3) as gip,
                tc.tile_pool(name="p1psT", bufs=2, space="PSUM") as psTp,
                tc.tile_pool(name="p1psL", bufs=2, space="PSUM") as psLp,
                tc.tile_pool(name="p1psG", bufs=2, space="PSUM") as psGp,
            ):
                for b in range(NBLK):
                    xT = xTp.tile([128, 2, NSEQ, BLK], BF16)
                    for s in range(NSEQ):
                        xs = xsp.tile([128, HID], F32, name="xs", tag="xs")
                        nc.sync.dma_start(
                            out=xs[:, :], in_=x_d[s, BLK * b:BLK * (b + 1), :])
                        xsb = xsbp.tile([128, HID], BF16, name="xsb", tag="xsb")
                        nc.vector.tensor_copy(xsb[:, :], xs[:, :])
                        for hc in range(2):
                            pst = psTp.tile([128, 128], BF16, name="pst",
                                            tag="pst")
                            nc.tensor.transpose(
                                pst[:, :], xsb[:, hc * 128:(hc + 1) * 128],
                                ident_b[:, :])
                            nc.vector.tensor_copy(xT[:, hc, s, :], pst[:, :])
                    gT = gTp.tile([128, 2, 4, NSEQ * BLK], BF16)
                    for br in range(2):
                        for m in range(4):
                            for n in range(4):
                                psl = psLp.tile([128, 512], F32, name="psl",
                                                tag="psl")
                                for k in range(2):
                                    nc.tensor.matmul(
                                        psl[:, :],
                                        linW[:, br, k, m * 128:(m + 1) * 128],
                                        xT[:, k, 4 * n:4 * (n + 1), :],
                                        start=(k == 0), stop=(k == 1))
                                nc.scalar.activation(
                                    gT[:, br, m, n * 512:(n + 1) * 512],
                                    psl[:, :], AF.Gelu,
                                    bias=linb[:, br * 4 + m:br * 4 + m + 1],
                                    scale=1.0)
                    for br in range(2):
                        gi = gip.tile([128, 6, NSEQ, BLK], BF16, name="gi",
                                      tag="gi")
                        for m in range(6):
                            for n in range(4):
                                psg = psGp.tile([128, 512], F32, name="psg",
                                                tag="psg")
                                for k in range(4):
                                    nc.tensor.matmul(
                                        psg[:, :],
                                        Wih[:, br, k, m * 128:(m + 1) * 128],
                                        gT[:, br, k, n * 512:(n + 1) * 512],
                                        start=(k == 0), stop=(k == 3))
                                nc.vector.tensor_scalar_add(
                                    gi[:, m, 4 * n:4 * (n + 1), :], psg[:, :],
                                    gib[:, br * 6 + m:br * 6 + m + 1])
                        dst = (gif_d if br == 0 else gibk_d)
                        nc.sync.dma_start(out=dst[:, b, :, :, :],
                                          in_=gi[:, :, :, :])

            # ---------------- P2: the scan ----------------
            with (
                tc.tile_pool(name="p2gf", bufs=2) as gfp,
                tc.tile_pool(name="p2gb", bufs=2) as gbp,
                tc.tile_pool(name="p2fT", bufs=2) as fTp,
                tc.tile_pool(name="p2bT", bufs=2) as bTp,
                tc.tile_pool(name="p2ps", bufs=8, space="PSUM") as psSp,
                tc.tile_pool(name="p2g", bufs=8) as gatep,
            ):
                prev_out = [None, None]
                for kb in range(NBLK):
                    gf = gfp.tile([128, 6, NSEQ, BLK], BF16)
                    nc.sync.dma_start(out=gf[:, :, :, :],
                                      in_=gif_d[:, kb, :, :, :])
                    gb = gbp.tile([128, 6, NSEQ, BLK], BF16)
                    nc.sync.dma_start(out=gb[:, :, :, :],
                                      in_=gibk_d[:, NBLK - 1 - kb, :, :, :])
                    fT = fTp.tile([128, 2, NSEQ, BLK], BF16)
                    bT = bTp.tile([128, 2, NSEQ, BLK], BF16)
                    blk_in = [gf, gb]
                    blk_out = [fT, bT]
                    for tl in range(BLK):
                        cols = [tl, BLK - 1 - tl]
                        psG = [None, None]
                        hprev = [None, None]
                        for br in range(2):
                            col = cols[br]
                            if tl == 0:
                                if kb == 0:
                                    hp = hz[:, :, :]
                                else:
                                    pc = (BLK - 1) if br == 0 else 0
                                    hp = prev_out[br][:, :, :, pc]
                            else:
                                pc = col + (-1 if br == 0 else 1)
                                hp = blk_out[br][:, :, :, pc]
                            hprev[br] = hp
                            ps = psSp.tile([128, 6, NSEQ], F32, name="ps",
                                           tag="ps")
                            psG[br] = ps
                            for m in range(6):
                                for kk in range(2):
                                    nc.tensor.matmul(
                                        ps[:, m, :],
                                        Whh[:, br, kk, m * 128:(m + 1) * 128],
                                        hp[:, kk, :],
                                        start=(kk == 0), stop=(kk == 1))
                        for br in range(2):
                            col = cols[br]
                            gi = blk_in[br]
                            ps = psG[br]
                            trz = gatep.tile([128, 4, NSEQ], BF16, name="trz",
                                             tag=f"trz{br}")
                            nc.vector.tensor_add(
                                trz[:, :, :], ps[:, 0:4, :], gi[:, 0:4, :, col])
                            rzs = gatep.tile([128, 4, NSEQ], BF16, name="rzs",
                                             tag=f"rzs{br}")
                            nc.scalar.activation(rzs[:, :, :], trz[:, :, :],
                                                 AF.Sigmoid)
                            rh = gatep.tile([128, 2, NSEQ], BF16, name="rh",
                                            tag=f"rh{br}")
                            nc.vector.tensor_mul(
                                rh[:, :, :], rzs[:, 0:2, :], ps[:, 4:6, :])
                            nin = gatep.tile([128, 2, NSEQ], BF16, name="nin",
                                             tag=f"nin{br}")
                            nc.vector.tensor_add(
                                nin[:, :, :], rh[:, :, :], gi[:, 4:6, :, col])
                            n_t = gatep.tile([128, 2, NSEQ], BF16, name="n_t",
                                             tag=f"n{br}")
                            nc.scalar.activation(n_t[:, :, :], nin[:, :, :],
                                                 AF.Tanh)
                            d_t = gatep.tile([128, 2, NSEQ], BF16, name="d_t",
                                             tag=f"d{br}")
                            nc.vector.tensor_sub(
                                d_t[:, :, :], hprev[br], n_t[:, :, :])
                            zd = gatep.tile([128, 2, NSEQ], BF16, name="zd",
                                            tag=f"zd{br}")
                            nc.vector.tensor_mul(
                                zd[:, :, :], rzs[:, 2:4, :], d_t[:, :, :])
                            nc.vector.tensor_add(
                                blk_out[br][:, :, :, col], n_t[:, :, :],
                                zd[:, :, :])
                    nc.sync.dma_start(out=fT_d[:, kb, :, :, :],
                                      in_=fT[:, :, :, :])
                    nc.sync.dma_start(out=bT_d[:, NBLK - 1 - kb, :, :, :],
                                      in_=bT[:, :, :, :])
                    prev_out = [fT, bT]

            # ---------------- P3: proj + residual + LN ----------------
            with (
                tc.tile_pool(name="p3f", bufs=2) as fSp,
                tc.tile_pool(name="p3b", bufs=2) as bSp,
                tc.tile_pool(name="p3pT", bufs=2) as pTp,
                tc.tile_pool(name="p3xr", bufs=3) as xrp,
                tc.tile_pool(name="p3res", bufs=3) as resp,
                tc.tile_pool(name="p3ln", bufs=3) as lnp,
                tc.tile_pool(name="p3out", bufs=3) as outp,
                tc.tile_pool(name="p3psP", bufs=2, space="PSUM") as psPp,
                tc.tile_pool(name="p3psB", bufs=3, space="PSUM") as psBp,
            ):
                for pb in range(NBLK):
                    for sg in range(NSEQ // 4):
                        fS = fSp.tile([128, 2, 512], BF16, name="fS", tag="fS")
                        bS = bSp.tile([128, 2, 512], BF16, name="bS", tag="bS")
                        for hc in range(2):
                            nc.sync.dma_start(
                                out=fS[:, hc, :],
                                in_=fT_d[:, pb, hc, 4 * sg:4 * (sg + 1), :])
                            nc.sync.dma_start(
                                out=bS[:, hc, :],
                                in_=bT_d[:, pb, hc, 4 * sg:4 * (sg + 1), :])
                        pT = pTp.tile([128, 2, 512], F32, name="pT", tag="pT")
                        for m in range(2):
                            psp = psPp.tile([128, 512], F32, name="psp",
                                            tag="psp")
                            for kk in range(4):
                                rhs = fS[:, kk, :] if kk < 2 else bS[:, kk - 2, :]
                                nc.tensor.matmul(
                                    psp[:, :],
                                    proj[:, kk, m * 128:(m + 1) * 128],
                                    rhs, start=(kk == 0), stop=(kk == 3))
                            nc.vector.tensor_scalar_add(
                                pT[:, m, :], psp[:, :], projb[:, m:m + 1])
                        for tt in range(4):
                            s = 4 * sg + tt
                            xr = xrp.tile([128, HID], F32, name="xr", tag="xr")
                            nc.sync.dma_start(
                                out=xr[:, :],
                                in_=x_d[s, BLK * pb:BLK * (pb + 1), :])
                            res = resp.tile([128, HID], F32, name="res",
                                            tag="res")
                            for hc in range(2):
                                psb = psBp.tile([128, 128], F32, name="psb",
                                                tag="psb")
                                nc.tensor.transpose(
                                    psb[:, :],
                                    pT[:, hc, tt * 128:(tt + 1) * 128],
                                    ident_f[:, :])
                                nc.vector.tensor_add(
                                    res[:, hc * 128:(hc + 1) * 128], psb[:, :],
                                    xr[:, hc * 128:(hc + 1) * 128])
                            mu = lnp.tile([128, 1], F32, name="mu", tag="mu")
                            nc.vector.tensor_reduce(
                                mu[:, :], res[:, :], axis=mybir.AxisListType.X,
                                op=mybir.AluOpType.add)
                            nc.vector.tensor_scalar_mul(
                                mu[:, :], mu[:, :], 1.0 / HID)
                            ct = lnp.tile([128, HID], F32, name="ct", tag="ct")
                            nc.vector.tensor_scalar_sub(
                                ct[:, :], res[:, :], mu[:, :])
                            sq = lnp.tile([128, HID], F32, name="sq", tag="sq")
                            ssq = lnp.tile([128, 1], F32, name="ssq", tag="ssq")
                            nc.vector.scalar_tensor_tensor(
                                out=sq[:, :], in0=ct[:, :], scalar=1.0,
                                in1=ct[:, :], op0=mybir.AluOpType.mult,
                                op1=mybir.AluOpType.mult, accum_out=ssq[:, :])
                            v2 = lnp.tile([128, 1], F32, name="v2", tag="v2")
                            nc.vector.tensor_scalar(
                                out=v2[:, :], in0=ssq[:, :], scalar1=1.0 / HID,
                                scalar2=LN_EPS, op0=mybir.AluOpType.mult,
                                op1=mybir.AluOpType.add)
                            rv = lnp.tile([128, 1], F32, name="rv", tag="rv")
                            nc.vector.reciprocal(rv[:, :], v2[:, :])
                            rstd = lnp.tile([128, 1], F32, name="rstd",
                                            tag="rstd")
                            nc.scalar.activation(rstd[:, :], rv[:, :], AF.Sqrt)
                            nmr = lnp.tile([128, 1], F32, name="nmr", tag="nmr")
                            nc.vector.scalar_tensor_tensor(
                                out=nmr[:, :], in0=mu[:, :], scalar=-1.0,
                                in1=rstd[:, :], op0=mybir.AluOpType.mult,
                                op1=mybir.AluOpType.mult)
                            y1 = lnp.tile([128, HID], F32, name="y1", tag="y1")
                            nc.scalar.activation(
                                y1[:, :], res[:, :], AF.Identity,
                                bias=nmr[:, :], scale=rstd[:, :])
                            y2 = lnp.tile([128, HID], F32, name="y2", tag="y2")
                            nc.vector.tensor_mul(y2[:, :], y1[:, :], lng[:, :])
                            ot = outp.tile([128, HID], F32, name="ot", tag="ot")
                            nc.vector.tensor_add(ot[:, :], y2[:, :], lnb[:, :])
                            nc.sync.dma_start(
                                out=out_d[s, BLK * pb:BLK * (pb + 1), :],
                                in_=ot[:, :])

    split_multi_waits(nc)
    return nc


# ---------------------------------------------------------------- host side
def _chunk_rows(w, nchunk):
    rows, cols = w.shape
    assert rows == nchunk * 128
    return np.ascontiguousarray(
        w.reshape(nchunk, 128, cols).transpose(1, 0, 2))


def _prep_inputs(kw):
    bf = ml_dtypes.bfloat16
    linW = np.stack([
        _chunk_rows(np.asarray(kw["fwd_lin_W"], np.float32), 2),
        _chunk_rows(np.asarray(kw["bwd_lin_W"], np.float32), 2)], axis=1)
    Wih = np.stack([
        _chunk_rows(np.asarray(kw["fwd_W_ih"], np.float32), 4),
        _chunk_rows(np.asarray(kw["bwd_W_ih"], np.float32), 4)], axis=1)
    Whh = np.stack([
        _chunk_rows(np.asarray(kw["fwd_W_hh"], np.float32), 2),
        _chunk_rows(np.asarray(kw["bwd_W_hh"], np.float32), 2)], axis=1)
    proj = _chunk_rows(np.asarray(kw["proj_W"], np.float32), 4)
    gibf = (np.asarray(kw["fwd_b_ih"], np.float32)
            + np.asarray(kw["fwd_b_hh"], np.float32))
    gibb = (np.asarray(kw["bwd_b_ih"], np.float32)
            + np.asarray(kw["bwd_b_hh"], np.float32))
    gib = np.concatenate([gibf.reshape(6, 128).T, gibb.reshape(6, 128).T],
                         axis=1)
    linb = np.concatenate(
        [np.asarray(kw["fwd_lin_b"], np.float32).reshape(4, 128).T,
         np.asarray(kw["bwd_lin_b"], np.float32).reshape(4, 128).T], axis=1)
    projb = np.asarray(kw["proj_b"], np.float32).reshape(2, 128).T
    lng = np.tile(np.asarray(kw["ln_g"], np.float32)[None, :], (128, 1))
    lnb = np.tile(np.asarray(kw["ln_b"], np.float32)[None, :], (128, 1))
    shared = {
        "linW": np.ascontiguousarray(linW.astype(bf)),
        "Wih": np.ascontiguousarray(Wih.astype(bf)),
        "Whh": np.ascontiguousarray(Whh.astype(bf)),
        "proj": np.ascontiguousarray(proj.astype(bf)),
        "gib": np.ascontiguousarray(gib),
        "linb": np.ascontiguousarray(linb),
        "projb": np.ascontiguousarray(projb),
        "lng": lng, "lnb": lnb,
    }
    x = np.asarray(kw["x"], np.float32)
    B, T, Pp, H = x.shape
    xf = x.reshape(B * T, Pp, H)
    in_maps = []
    for c in range(NC):
        m = dict(shared)
        m["x"] = np.ascontiguousarray(xf[NSEQ * c:NSEQ * (c + 1)])
        in_maps.append(m)
    return in_maps, (B, T, Pp, H)


def get_runner():
    if "r" not in _runner_cache:
        nc = build_nc()
        _runner_cache["r"] = SpmdRunner(nc, n_cores=NC)
    return _runner_cache["r"]


def kernel(**inputs):
    in_maps, (B, T, Pp, H) = _prep_inputs(inputs)
    r = get_runner()
    res = r.run(in_maps)
    out = np.empty((B * T, Pp, H), np.float32)
    for c in range(NC):
        out[NSEQ * c:NSEQ * (c + 1)] = res[c]["out"]
    return out.reshape(B, T, Pp, H)

